# revision 1
# baseline (speedup 1.0000x reference)
"""GIN-style GNN (2 layers) on 8 NeuronCores, node-parallel by destination.

Host does integer index prep only: append self-loops, bucket+sort edges by
dst into per-core / per-128-node-tile chunks of 128 edges (padded), and a
per-node histogram of the 21 edge-attr classes. All floating-point math runs
on device via three SPMD launches:
  L2: h0 embedding gather, layer-0 aggregation (one-hot matmul segment-sum),
      MLP, partial BN stats.
  L3: BN0 apply + relu (full, replicated) -> row-major h1, layer-1 agg+MLP,
      partial BN stats.
  L4: BN1 apply on the local node slice -> row-major output.
"""

import sys

sys.path.insert(0, "/opt/trn_rl_repo")

import numpy as np

import concourse.bass as bass
import concourse.tile as tile
from concourse import bacc, mybir
from concourse.bass_utils import run_bass_kernel_spmd
from concourse.masks import make_identity

N = 50000
E = 800000
D = 128
P = 128
NCORES = 8
NPC = N // NCORES          # 6250 nodes per core
NT = (NPC + P - 1) // P    # 49 output tiles per core (last has 106 rows)
BN_EPS = 1e-5
F32 = mybir.dt.float32
I32 = mybir.dt.int32


def _pack_cols(arr2d):
    """[n_chunks*128] flat per-chunk values -> SBUF layout [128, n_chunks]."""
    n = arr2d.shape[0]
    return np.ascontiguousarray(arr2d.reshape(n // P, P).T)


def _host_prep(x, edge_index, edge_attr):
    """Pure integer preprocessing. Returns per-core index arrays and K."""
    x = np.asarray(x)
    ei = np.asarray(edge_index)
    ea = np.asarray(edge_attr)

    loop = np.arange(N, dtype=np.int64)
    src = np.concatenate([ei[0], loop]).astype(np.int64)
    dst = np.concatenate([ei[1], loop]).astype(np.int64)
    t = np.concatenate([ea[:, 0] * 3 + ea[:, 1], np.full(N, 4 * 3, np.int64)])

    per_core = []
    counts_all = []
    for c in range(NCORES):
        lo, hi = c * NPC, (c + 1) * NPC
        m = (dst >= lo) & (dst < hi)
        es, ed, et = src[m], dst[m] - lo, t[m]
        order = np.argsort(ed, kind="stable")
        es, ed, et = es[order], ed[order], et[order]
        # per-tile edge ranges via searchsorted on the sorted dst-local
        bounds = np.searchsorted(ed, np.arange(0, NPC + P, P))
        per_core.append((es, ed, et, bounds))
        cnts = bounds[1:NT + 1] - bounds[:NT]
        counts_all.append(cnts)
    K = int(np.max([np.ceil(c / P) for c in np.concatenate(counts_all)]))

    packed = []
    for c in range(NCORES):
        es, ed, et, bounds = per_core[c]
        srcg = np.zeros((NT, K * P), np.int32)
        dstg = np.full((NT, K * P), 999.0, np.float32)
        cntT = np.zeros((NPC, 21), np.float32)
        np.add.at(cntT, (ed, et), 1.0)
        for ti in range(NT):
            a, b = bounds[ti], bounds[ti + 1]
            n = b - a
            srcg[ti, :n] = es[a:b]
            dstg[ti, :n] = (ed[a:b] - ti * P).astype(np.float32)
        packed.append({
            "srcp": _pack_cols(srcg.reshape(-1)),          # [128, NT*K] i32
            "dstp": _pack_cols(dstg.reshape(-1)).astype(np.float32),
            "cntT": np.ascontiguousarray(cntT.T),          # [21, NPC] f32
        })
    return packed, K


def _load_const(nc, pool, dram_ap, shape, dtype):
    sb = pool.tile(shape, dtype, name=f"c_{dram_ap.name}")
    nc.sync.dma_start(out=sb[:], in_=dram_ap[:])
    return sb


def _layer_body(nc, tc, ctx, K, *, h_rows, srcp, dstp, cntT, e1r, e2t,
                w1, w2a, w2b, b1a, b1b, b2, iota, hout_T, stats_out):
    """Shared agg + MLP + stats body (one GNN layer) on the core's node slice."""
    const = ctx.enter_context(tc.tile_pool(name="const", bufs=1))
    work = ctx.enter_context(tc.tile_pool(name="work", bufs=4))
    psA = ctx.enter_context(tc.tile_pool(name="psA", bufs=1, space="PSUM"))
    psB = ctx.enter_context(tc.tile_pool(name="psB", bufs=2, space="PSUM"))
    psC = ctx.enter_context(tc.tile_pool(name="psC", bufs=1, space="PSUM"))
    accp = ctx.enter_context(tc.tile_pool(name="accp", bufs=1))

    srcp_sb = _load_const(nc, const, srcp, [P, NT * K], I32)
    dstp_sb = _load_const(nc, const, dstp, [P, NT * K], F32)
    cnt_sb = _load_const(nc, const, cntT, [21, NPC], F32)
    iota_sb = _load_const(nc, const, iota, [P, P], F32)
    e1r_sb = _load_const(nc, const, e1r, [21, D], F32)
    e2t_sb = _load_const(nc, const, e2t, [21, D], F32)
    w1_sb = _load_const(nc, const, w1, [D, 2 * D], F32)
    w2a_sb = _load_const(nc, const, w2a, [D, D], F32)
    w2b_sb = _load_const(nc, const, w2b, [D, D], F32)
    b1a_sb = _load_const(nc, const, b1a, [D, 1], F32)
    b1b_sb = _load_const(nc, const, b1b, [D, 1], F32)
    b2_sb = _load_const(nc, const, b2, [D, 1], F32)

    etab = const.tile([21, D], F32)
    nc.vector.tensor_add(etab[:], e1r_sb[:], e2t_sb[:])

    s1_acc = accp.tile([P, 1], F32)
    s2_acc = accp.tile([P, 1], F32)
    nc.vector.memset(s1_acc[:], 0.0)
    nc.vector.memset(s2_acc[:], 0.0)

    for ti in range(NT):
        cols = min(P, NPC - ti * P)
        agg_ps = psA.tile([P, P], F32, space="PSUM")
        # edge-embedding term: aggT[d,n] += sum_k etab[k,d] * cntT[k,n]
        nc.tensor.matmul(
            out=agg_ps[:, :cols], lhsT=etab[:],
            rhs=cnt_sb[:, ti * P:ti * P + cols],
            start=True, stop=False, skip_group_check=True)
        for j in range(K):
            col = ti * K + j
            hg = work.tile([P, D], F32)
            nc.gpsimd.indirect_dma_start(
                out=hg[:], out_offset=None, in_=h_rows[:],
                in_offset=bass.IndirectOffsetOnAxis(
                    ap=srcp_sb[:, col:col + 1], axis=0))
            oh = work.tile([P, P], F32)
            nc.vector.tensor_tensor(
                out=oh[:, :cols],
                in0=dstp_sb[:, col:col + 1].to_broadcast([P, cols]),
                in1=iota_sb[:, :cols], op=mybir.AluOpType.is_equal)
            nc.tensor.matmul(
                out=agg_ps[:, :cols], lhsT=hg[:], rhs=oh[:, :cols],
                start=False, stop=(j == K - 1), skip_group_check=True)
        aggT = work.tile([P, P], F32)
        nc.vector.tensor_copy(out=aggT[:, :cols], in_=agg_ps[:, :cols])

        # z1T = W1^T @ agg  (two 128-row chunks of the 256 hidden units)
        r = []
        for half, bsb in ((0, b1a_sb), (1, b1b_sb)):
            z_ps = psB.tile([P, P], F32, space="PSUM")
            nc.tensor.matmul(
                out=z_ps[:, :cols], lhsT=w1_sb[:, half * D:(half + 1) * D],
                rhs=aggT[:, :cols], start=True, stop=True,
                skip_group_check=True)
            rh = work.tile([P, P], F32)
            nc.vector.tensor_tensor(
                out=rh[:, :cols], in0=z_ps[:, :cols],
                in1=bsb[:, :1].to_broadcast([P, cols]),
                op=mybir.AluOpType.add)
            nc.vector.tensor_scalar_max(rh[:, :cols], rh[:, :cols], 0.0)
            r.append(rh)

        h2_ps = psC.tile([P, P], F32, space="PSUM")
        nc.tensor.matmul(out=h2_ps[:, :cols], lhsT=w2a_sb[:], rhs=r[0][:, :cols],
                         start=True, stop=False, skip_group_check=True)
        nc.tensor.matmul(out=h2_ps[:, :cols], lhsT=w2b_sb[:], rhs=r[1][:, :cols],
                         start=False, stop=True, skip_group_check=True)
        h2t = work.tile([P, P], F32)
        nc.vector.tensor_tensor(
            out=h2t[:, :cols], in0=h2_ps[:, :cols],
            in1=b2_sb[:, :1].to_broadcast([P, cols]), op=mybir.AluOpType.add)
        nc.sync.dma_start(out=hout_T[:, ti * P:ti * P + cols],
                          in_=h2t[:, :cols])
        # partial BN stats over this tile's nodes (free-axis reductions)
        part = work.tile([P, 1], F32)
        nc.vector.reduce_sum(out=part[:], in_=h2t[:, :cols],
                             axis=mybir.AxisListType.X)
        nc.vector.tensor_add(s1_acc[:], s1_acc[:], part[:])
        sq = work.tile([P, P], F32)
        nc.vector.tensor_mul(sq[:, :cols], h2t[:, :cols], h2t[:, :cols])
        part2 = work.tile([P, 1], F32)
        nc.vector.reduce_sum(out=part2[:], in_=sq[:, :cols],
                             axis=mybir.AxisListType.X)
        nc.vector.tensor_add(s2_acc[:], s2_acc[:], part2[:])

    nc.sync.dma_start(out=stats_out[:, 0:1], in_=s1_acc[:])
    nc.sync.dma_start(out=stats_out[:, 1:2], in_=s2_acc[:])


def _bn_coeffs(nc, pool, stats_sb, gamma_sb, beta_sb):
    """a = gamma*rsqrt(var+eps), b = beta - a*mu from 8 partial (s1,s2)."""
    mu = pool.tile([P, 1], F32)
    nc.vector.reduce_sum(out=mu[:], in_=stats_sb[:, 0:NCORES],
                         axis=mybir.AxisListType.X)
    nc.vector.tensor_scalar_mul(mu[:], mu[:], 1.0 / N)
    ex2 = pool.tile([P, 1], F32)
    nc.vector.reduce_sum(out=ex2[:], in_=stats_sb[:, NCORES:2 * NCORES],
                         axis=mybir.AxisListType.X)
    nc.vector.tensor_scalar_mul(ex2[:], ex2[:], 1.0 / N)
    var = pool.tile([P, 1], F32)
    nc.vector.tensor_mul(var[:], mu[:], mu[:])
    nc.vector.tensor_tensor(out=var[:], in0=ex2[:], in1=var[:],
                            op=mybir.AluOpType.subtract)
    nc.vector.tensor_scalar_add(var[:], var[:], BN_EPS)
    std = pool.tile([P, 1], F32)
    nc.scalar.activation(out=std[:], in_=var[:],
                         func=mybir.ActivationFunctionType.Sqrt)
    rstd = pool.tile([P, 1], F32)
    nc.vector.reciprocal(out=rstd[:], in_=std[:])
    a = pool.tile([P, 1], F32)
    nc.vector.tensor_mul(a[:], gamma_sb[:], rstd[:])
    b = pool.tile([P, 1], F32)
    nc.vector.tensor_mul(b[:], a[:], mu[:])
    nc.vector.tensor_tensor(out=b[:], in0=beta_sb[:], in1=b[:],
                            op=mybir.AluOpType.subtract)
    return a, b


def _build_l2(K):
    nc = bacc.Bacc(None, target_bir_lowering=False)
    x0p = nc.dram_tensor("x0p", [P, (N + P - 1) // P], I32, kind="ExternalInput")
    x1p = nc.dram_tensor("x1p", [P, (N + P - 1) // P], I32, kind="ExternalInput")
    xe1 = nc.dram_tensor("xe1", [120, D], F32, kind="ExternalInput")
    xe2 = nc.dram_tensor("xe2", [3, D], F32, kind="ExternalInput")
    srcp = nc.dram_tensor("srcp", [P, NT * K], I32, kind="ExternalInput")
    dstp = nc.dram_tensor("dstp", [P, NT * K], F32, kind="ExternalInput")
    cntT = nc.dram_tensor("cntT", [21, NPC], F32, kind="ExternalInput")
    e1r = nc.dram_tensor("e1r", [21, D], F32, kind="ExternalInput")
    e2t = nc.dram_tensor("e2t", [21, D], F32, kind="ExternalInput")
    w1 = nc.dram_tensor("w1", [D, 2 * D], F32, kind="ExternalInput")
    w2a = nc.dram_tensor("w2a", [D, D], F32, kind="ExternalInput")
    w2b = nc.dram_tensor("w2b", [D, D], F32, kind="ExternalInput")
    b1a = nc.dram_tensor("b1a", [D, 1], F32, kind="ExternalInput")
    b1b = nc.dram_tensor("b1b", [D, 1], F32, kind="ExternalInput")
    b2 = nc.dram_tensor("b2", [D, 1], F32, kind="ExternalInput")
    iota = nc.dram_tensor("iota", [P, P], F32, kind="ExternalInput")
    h2T = nc.dram_tensor("h2T", [P, NPC], F32, kind="ExternalOutput")
    stats = nc.dram_tensor("stats", [P, 2], F32, kind="ExternalOutput")
    h0 = nc.dram_tensor("h0", [N, D], F32)

    from contextlib import ExitStack
    with tile.TileContext(nc) as tc, ExitStack() as ctx:
        pool = ctx.enter_context(tc.tile_pool(name="h0c", bufs=1))
        wp = ctx.enter_context(tc.tile_pool(name="h0w", bufs=4))
        x0_sb = _load_const(nc, pool, x0p, [P, (N + P - 1) // P], I32)
        x1_sb = _load_const(nc, pool, x1p, [P, (N + P - 1) // P], I32)
        nch = (N + P - 1) // P
        for ci in range(nch):
            rows = min(P, N - ci * P)
            ga = wp.tile([P, D], F32)
            nc.gpsimd.indirect_dma_start(
                out=ga[:], out_offset=None, in_=xe1[:],
                in_offset=bass.IndirectOffsetOnAxis(
                    ap=x0_sb[:, ci:ci + 1], axis=0))
            gb = wp.tile([P, D], F32)
            nc.gpsimd.indirect_dma_start(
                out=gb[:], out_offset=None, in_=xe2[:],
                in_offset=bass.IndirectOffsetOnAxis(
                    ap=x1_sb[:, ci:ci + 1], axis=0))
            hs = wp.tile([P, D], F32)
            nc.vector.tensor_add(hs[:], ga[:], gb[:])
            nc.sync.dma_start(out=h0[ci * P:ci * P + rows, :],
                              in_=hs[:rows, :])
        _layer_body(nc, tc, ctx, K, h_rows=h0, srcp=srcp, dstp=dstp,
                    cntT=cntT, e1r=e1r, e2t=e2t, w1=w1, w2a=w2a, w2b=w2b,
                    b1a=b1a, b1b=b1b, b2=b2, iota=iota, hout_T=h2T,
                    stats_out=stats)
    nc.compile()
    return nc


def _build_l3(K):
    nc = bacc.Bacc(None, target_bir_lowering=False)
    h2Tf = nc.dram_tensor("h2Tf", [P, N], F32, kind="ExternalInput")
    statsA = nc.dram_tensor("statsA", [P, 2 * NCORES], F32, kind="ExternalInput")
    gamma = nc.dram_tensor("gamma", [D, 1], F32, kind="ExternalInput")
    beta = nc.dram_tensor("beta", [D, 1], F32, kind="ExternalInput")
    srcp = nc.dram_tensor("srcp", [P, NT * K], I32, kind="ExternalInput")
    dstp = nc.dram_tensor("dstp", [P, NT * K], F32, kind="ExternalInput")
    cntT = nc.dram_tensor("cntT", [21, NPC], F32, kind="ExternalInput")
    e1r = nc.dram_tensor("e1r", [21, D], F32, kind="ExternalInput")
    e2t = nc.dram_tensor("e2t", [21, D], F32, kind="ExternalInput")
    w1 = nc.dram_tensor("w1", [D, 2 * D], F32, kind="ExternalInput")
    w2a = nc.dram_tensor("w2a", [D, D], F32, kind="ExternalInput")
    w2b = nc.dram_tensor("w2b", [D, D], F32, kind="ExternalInput")
    b1a = nc.dram_tensor("b1a", [D, 1], F32, kind="ExternalInput")
    b1b = nc.dram_tensor("b1b", [D, 1], F32, kind="ExternalInput")
    b2 = nc.dram_tensor("b2", [D, 1], F32, kind="ExternalInput")
    iota = nc.dram_tensor("iota", [P, P], F32, kind="ExternalInput")
    h3T = nc.dram_tensor("h3T", [P, NPC], F32, kind="ExternalOutput")
    stats = nc.dram_tensor("stats", [P, 2], F32, kind="ExternalOutput")
    h1 = nc.dram_tensor("h1", [N, D], F32)

    from contextlib import ExitStack
    with tile.TileContext(nc) as tc, ExitStack() as ctx:
        cpool = ctx.enter_context(tc.tile_pool(name="bnc", bufs=1))
        wp = ctx.enter_context(tc.tile_pool(name="bnw", bufs=4))
        pp = ctx.enter_context(tc.tile_pool(name="bnp", bufs=4, space="PSUM"))
        st_sb = _load_const(nc, cpool, statsA, [P, 2 * NCORES], F32)
        g_sb = _load_const(nc, cpool, gamma, [D, 1], F32)
        be_sb = _load_const(nc, cpool, beta, [D, 1], F32)
        ident = cpool.tile([P, P], F32)
        make_identity(nc, ident[:])
        a, b = _bn_coeffs(nc, cpool, st_sb, g_sb, be_sb)
        nch = (N + P - 1) // P
        for ci in range(nch):
            rows = min(P, N - ci * P)
            xt = wp.tile([P, P], F32)
            nc.sync.dma_start(out=xt[:, :rows],
                              in_=h2Tf[:, ci * P:ci * P + rows])
            nc.vector.tensor_tensor(out=xt[:, :rows], in0=xt[:, :rows],
                                    in1=a[:, :1].to_broadcast([P, rows]),
                                    op=mybir.AluOpType.mult)
            nc.vector.tensor_tensor(out=xt[:, :rows], in0=xt[:, :rows],
                                    in1=b[:, :1].to_broadcast([P, rows]),
                                    op=mybir.AluOpType.add)
            nc.vector.tensor_scalar_max(xt[:, :rows], xt[:, :rows], 0.0)
            tp = pp.tile([P, P], F32, space="PSUM")
            nc.tensor.transpose(out=tp[:rows, :], in_=xt[:, :rows],
                                identity=ident[:])
            hrow = wp.tile([P, D], F32)
            nc.vector.tensor_copy(out=hrow[:rows, :], in_=tp[:rows, :])
            nc.sync.dma_start(out=h1[ci * P:ci * P + rows, :],
                              in_=hrow[:rows, :])
        _layer_body(nc, tc, ctx, K, h_rows=h1, srcp=srcp, dstp=dstp,
                    cntT=cntT, e1r=e1r, e2t=e2t, w1=w1, w2a=w2a, w2b=w2b,
                    b1a=b1a, b1b=b1b, b2=b2, iota=iota, hout_T=h3T,
                    stats_out=stats)
    nc.compile()
    return nc


def _build_l4():
    nc = bacc.Bacc(None, target_bir_lowering=False)
    h3T = nc.dram_tensor("h3T", [P, NPC], F32, kind="ExternalInput")
    statsA = nc.dram_tensor("statsA", [P, 2 * NCORES], F32, kind="ExternalInput")
    gamma = nc.dram_tensor("gamma", [D, 1], F32, kind="ExternalInput")
    beta = nc.dram_tensor("beta", [D, 1], F32, kind="ExternalInput")
    outr = nc.dram_tensor("outr", [NPC, D], F32, kind="ExternalOutput")

    from contextlib import ExitStack
    with tile.TileContext(nc) as tc, ExitStack() as ctx:
        cpool = ctx.enter_context(tc.tile_pool(name="c", bufs=1))
        wp = ctx.enter_context(tc.tile_pool(name="w", bufs=4))
        pp = ctx.enter_context(tc.tile_pool(name="p", bufs=4, space="PSUM"))
        st_sb = _load_const(nc, cpool, statsA, [P, 2 * NCORES], F32)
        g_sb = _load_const(nc, cpool, gamma, [D, 1], F32)
        be_sb = _load_const(nc, cpool, beta, [D, 1], F32)
        ident = cpool.tile([P, P], F32)
        make_identity(nc, ident[:])
        a, b = _bn_coeffs(nc, cpool, st_sb, g_sb, be_sb)
        for ti in range(NT):
            cols = min(P, NPC - ti * P)
            xt = wp.tile([P, P], F32)
            nc.sync.dma_start(out=xt[:, :cols],
                              in_=h3T[:, ti * P:ti * P + cols])
            nc.vector.tensor_tensor(out=xt[:, :cols], in0=xt[:, :cols],
                                    in1=a[:, :1].to_broadcast([P, cols]),
                                    op=mybir.AluOpType.mult)
            nc.vector.tensor_tensor(out=xt[:, :cols], in0=xt[:, :cols],
                                    in1=b[:, :1].to_broadcast([P, cols]),
                                    op=mybir.AluOpType.add)
            tp = pp.tile([P, P], F32, space="PSUM")
            nc.tensor.transpose(out=tp[:cols, :], in_=xt[:, :cols],
                                identity=ident[:])
            orow = wp.tile([P, D], F32)
            nc.vector.tensor_copy(out=orow[:cols, :], in_=tp[:cols, :])
            nc.sync.dma_start(out=outr[ti * P:ti * P + cols, :],
                              in_=orow[:cols, :])
    nc.compile()
    return nc


LAUNCH_NS = []


def _run(nc, maps, cores):
    import time as _t
    t0 = _t.monotonic_ns()
    res = run_bass_kernel_spmd(nc, maps, cores)
    dt = _t.monotonic_ns() - t0
    LAUNCH_NS.append(res.exec_time_ns if res.exec_time_ns else dt)
    return res


def kernel(x, edge_index, edge_attr, batch, xemb1, xemb2, e1, e2,
           W1, b1, W2, b2, gamma, beta):
    LAUNCH_NS.clear()
    packed, K = _host_prep(x, edge_index, edge_attr)
    f32 = np.float32
    nchp = (N + P - 1) // P
    x0 = np.zeros(nchp * P, np.int32)
    x0[:N] = np.asarray(x)[:, 0]
    x1 = np.zeros(nchp * P, np.int32)
    x1[:N] = np.asarray(x)[:, 1]
    x0p, x1p = _pack_cols(x0), _pack_cols(x1)
    iota = np.broadcast_to(np.arange(P, dtype=f32), (P, P)).copy()

    def wdict(l):
        return {
            "e1r": np.repeat(np.asarray(e1[l], f32), 3, axis=0).copy(),
            "e2t": np.tile(np.asarray(e2[l], f32), (7, 1)).copy(),
            "w1": np.asarray(W1[l], f32).copy(),
            "w2a": np.asarray(W2[l][:D], f32).copy(),
            "w2b": np.asarray(W2[l][D:], f32).copy(),
            "b1a": np.asarray(b1[l][:D], f32).reshape(D, 1).copy(),
            "b1b": np.asarray(b1[l][D:], f32).reshape(D, 1).copy(),
            "b2": np.asarray(b2[l], f32).reshape(D, 1).copy(),
            "iota": iota,
        }

    cores = list(range(NCORES))
    w0, w1d = wdict(0), wdict(1)

    nc2 = _build_l2(K)
    maps = []
    for c in cores:
        m = {"x0p": x0p, "x1p": x1p,
             "xe1": np.asarray(xemb1, f32).copy(),
             "xe2": np.asarray(xemb2, f32).copy(),
             "srcp": packed[c]["srcp"], "dstp": packed[c]["dstp"],
             "cntT": packed[c]["cntT"]}
        m.update(w0)
        maps.append(m)
    res2 = _run(nc2, maps, cores).results

    h2Tf = np.concatenate([r["h2T"] for r in res2], axis=1)
    statsA0 = np.concatenate([r["stats"] for r in res2], axis=1)
    statsA0 = np.concatenate([statsA0[:, 0::2], statsA0[:, 1::2]], axis=1)

    nc3 = _build_l3(K)
    maps = []
    for c in cores:
        m = {"h2Tf": h2Tf, "statsA": statsA0,
             "gamma": np.asarray(gamma[0], f32).reshape(D, 1).copy(),
             "beta": np.asarray(beta[0], f32).reshape(D, 1).copy(),
             "srcp": packed[c]["srcp"], "dstp": packed[c]["dstp"],
             "cntT": packed[c]["cntT"]}
        m.update(w1d)
        maps.append(m)
    res3 = _run(nc3, maps, cores).results

    statsA1 = np.concatenate([r["stats"] for r in res3], axis=1)
    statsA1 = np.concatenate([statsA1[:, 0::2], statsA1[:, 1::2]], axis=1)

    nc4 = _build_l4()
    maps = []
    for c in cores:
        maps.append({"h3T": res3[c]["h3T"], "statsA": statsA1,
                     "gamma": np.asarray(gamma[1], f32).reshape(D, 1).copy(),
                     "beta": np.asarray(beta[1], f32).reshape(D, 1).copy()})
    res4 = _run(nc4, maps, cores).results

    return np.concatenate([r["outr"] for r in res4], axis=0)



# revision 2
# speedup vs baseline: 1.2250x; 1.2250x over previous
"""GIN-style GNN (2 layers) fused into ONE SPMD launch on 8 NeuronCores.

Node-parallel by destination: core c owns nodes [c*6250, (c+1)*6250). Host
does integer index prep only, packed in narrow dtypes (u8/u16) and few
tensors to minimize tunnel transfer and per-buffer overhead.

Device program per core:
  h0 slice (embedding gathers from combined table) -> AllGather -> h0_full
  per layer: segment-sum aggregation (gather + one-hot matmul, feature-major)
  -> MLP where the last matmul flips stationary/moving so h2 lands row-major
  -> BN stats as ones-vector matmuls -> AllReduce -> BN apply on rows
  (no transposes anywhere) -> h1 slice -> AllGather -> layer 2 -> bf16 out.
"""

import sys

sys.path.insert(0, "/opt/trn_rl_repo")

from contextlib import ExitStack

import numpy as np

import concourse.bass as bass
import concourse.tile as tile
from concourse import bacc, mybir
from concourse.bass_utils import run_bass_kernel_spmd

N = 50000
E = 800000
D = 128
P = 128
NCORES = 8
NPC = N // NCORES          # 6250 nodes per core
NT = (NPC + P - 1) // P    # 49 tiles per core (last has 106 rows)
BN_EPS = 1e-5
F32 = mybir.dt.float32
I32 = mybir.dt.int32
U8 = mybir.dt.uint8
U16 = mybir.dt.uint16
BF16 = mybir.dt.bfloat16

# pf32 column layout: iota | w1_0 | w1_1 | w2a_0 | w2b_0 | w2a_1 | w2b_1 |
#                     etab0 | etab1 | b1a_0 b1b_0 b1a_1 b1b_1
C_IOTA = 0
C_W1 = (128, 128 + 256)                    # per layer l: C_W1[l]
C_W2 = (640, 768, 896, 1024)               # w2a_0 w2b_0 w2a_1 w2b_1
C_ET = (1152, 1280)
C_B1 = 1408                                # 4 columns
NF32 = 1412
# vrow column layout: b2_0 | b2_1 | gam0 | bet0 | gam1 | bet1
NVR = 6 * 128


def _pack_cols(flat):
    n = flat.shape[0]
    return np.ascontiguousarray(flat.reshape(n // P, P).T)


def _host_prep(x, edge_index, edge_attr):
    """Integer preprocessing -> per-core packed arrays + chunk layout."""
    x = np.asarray(x)
    ei = np.asarray(edge_index)
    ea = np.asarray(edge_attr)

    loop = np.arange(N, dtype=np.int64)
    src = np.concatenate([ei[0], loop]).astype(np.int64)
    dst = np.concatenate([ei[1], loop]).astype(np.int64)
    t = np.concatenate([ea[:, 0] * 3 + ea[:, 1], np.full(N, 4 * 3, np.int64)])

    per_core = []
    cnts = np.zeros((NCORES, NT), np.int64)
    for c in range(NCORES):
        lo = c * NPC
        m = (dst >= lo) & (dst < lo + NPC)
        es, ed, et = src[m], dst[m] - lo, t[m]
        order = np.argsort(ed, kind="stable")
        es, ed, et = es[order], ed[order], et[order]
        bounds = np.searchsorted(ed, np.arange(0, NPC + P, P))
        per_core.append((es, ed, et, bounds))
        cnts[c] = bounds[1:NT + 1] - bounds[:NT]
    kt = np.ceil(cnts.max(axis=0) / P).astype(np.int64)   # per-tile chunks
    co = np.concatenate([[0], np.cumsum(kt)])             # column offsets
    ct = int(co[-1])                                      # total chunk cols

    packed = []
    for c in range(NCORES):
        es, ed, et, bounds = per_core[c]
        srcg = np.zeros((ct, P), np.uint16)
        dstg = np.full((ct, P), 255, np.uint8)
        cntT = np.zeros((NPC, 21), np.uint8)
        np.add.at(cntT, (ed, et), 1)
        for ti in range(NT):
            a, b = bounds[ti], bounds[ti + 1]
            n = b - a
            blk_s = srcg[co[ti]:co[ti + 1]].reshape(-1)
            blk_s[:n] = es[a:b].astype(np.uint16)
            srcg[co[ti]:co[ti + 1]] = blk_s.reshape(kt[ti], P)
            blk_d = dstg[co[ti]:co[ti + 1]].reshape(-1)
            blk_d[:n] = (ed[a:b] - ti * P).astype(np.uint8)
            dstg[co[ti]:co[ti + 1]] = blk_d.reshape(kt[ti], P)
        xs = np.zeros((2, NT * P), np.uint8)
        xs[0, :NPC] = x[c * NPC:(c + 1) * NPC, 0]
        xs[1, :NPC] = x[c * NPC:(c + 1) * NPC, 1] + 120   # xe2 rows offset
        packed.append({
            "sp": np.ascontiguousarray(srcg.T),            # [128, ct] u16
            "dp": np.ascontiguousarray(dstg.T),            # [128, ct] u8
            "cn": np.ascontiguousarray(cntT.T),            # [21, NPC] u8
            "xp": np.concatenate([_pack_cols(xs[0]), _pack_cols(xs[1])],
                                 axis=1),                  # [128, 2*NT] u8
        })
    return packed, kt.tolist(), co.tolist(), ct


def _cast_chunked(nc, pool, src_sb, shape, dtype, name, chunk=512):
    dst = pool.tile(shape, dtype, name=name)
    p, n = shape
    for c0 in range(0, n, chunk):
        c1 = min(n, c0 + chunk)
        nc.vector.tensor_copy(out=dst[:p, c0:c1], in_=src_sb[:p, c0:c1])
    return dst


def _build(kt, co, ct):
    nc = bacc.Bacc(None, target_bir_lowering=False,
                   disable_frame_to_traceback=True)
    tb = nc.dram_tensor("tb", [123, D], F32, kind="ExternalInput")
    pf = nc.dram_tensor("pf", [P, NF32], F32, kind="ExternalInput")
    vr = nc.dram_tensor("vr", [1, NVR], F32, kind="ExternalInput")
    sp = nc.dram_tensor("sp", [P, ct], U16, kind="ExternalInput")
    dp = nc.dram_tensor("dp", [P, ct], U8, kind="ExternalInput")
    cn = nc.dram_tensor("cn", [21, NPC], U8, kind="ExternalInput")
    xp = nc.dram_tensor("xp", [P, 2 * NT], U8, kind="ExternalInput")
    outr = nc.dram_tensor("o", [NPC, D], BF16, kind="ExternalOutput")

    RG = [list(range(NCORES))]
    with tile.TileContext(nc) as tc, ExitStack() as ctx:
        cp = ctx.enter_context(tc.tile_pool(name="c", bufs=1))
        wp = ctx.enter_context(tc.tile_pool(name="w", bufs=4))
        pa = ctx.enter_context(tc.tile_pool(name="pa", bufs=2, space="PSUM"))
        pb = ctx.enter_context(tc.tile_pool(name="pb", bufs=2, space="PSUM"))
        pc = ctx.enter_context(tc.tile_pool(name="pc", bufs=2, space="PSUM"))
        pst = ctx.enter_context(tc.tile_pool(name="pt", bufs=1, space="PSUM"))
        dr = ctx.enter_context(tc.tile_pool(name="d", bufs=1, space="DRAM"))

        pf_sb = cp.tile([P, NF32], F32)
        nc.sync.dma_start(out=pf_sb[:], in_=pf[:])
        vr_sb = cp.tile([1, NVR], F32)
        nc.sync.dma_start(out=vr_sb[:], in_=vr[:])
        sp_u = cp.tile([P, ct], U16)
        nc.sync.dma_start(out=sp_u[:], in_=sp[:])
        dp_u = cp.tile([P, ct], U8)
        nc.sync.dma_start(out=dp_u[:], in_=dp[:])
        cn_u = cp.tile([21, NPC], U8)
        nc.sync.dma_start(out=cn_u[:], in_=cn[:])
        xp_u = cp.tile([P, 2 * NT], U8)
        nc.sync.dma_start(out=xp_u[:], in_=xp[:])

        spi = _cast_chunked(nc, cp, sp_u, [P, ct], I32, "spi")
        dpf = _cast_chunked(nc, cp, dp_u, [P, ct], F32, "dpf")
        cnf = _cast_chunked(nc, cp, cn_u, [21, NPC], F32, "cnf")
        xpi = _cast_chunked(nc, cp, xp_u, [P, 2 * NT], I32, "xpi")

        iota = pf_sb[:, 0:128]
        ones = cp.tile([P, 1], F32)
        nc.vector.memset(ones[:], 1.0)
        onesr = cp.tile([1, P], F32)
        nc.vector.memset(onesr[:], 1.0)

        # replicate per-layer b2 row to [128, D] via rank-1 matmul
        b2r = []
        for l in range(2):
            rp = pc.tile([P, P], F32, space="PSUM", name="h2p")
            nc.tensor.matmul(out=rp[:], lhsT=onesr[:],
                             rhs=vr_sb[:, l * D:(l + 1) * D],
                             start=True, stop=True, skip_group_check=True)
            rs = cp.tile([P, P], F32, name=f"b2r{l}")
            nc.vector.tensor_copy(out=rs[:], in_=rp[:])
            b2r.append(rs)

        h2R = [cp.tile([P, NT * D], F32, name="h2R0"),
               cp.tile([P, NT * D], F32, name="h2R1")]

        # ---- h0 slice: gather node-type embeddings (combined table)
        h0s = dr.tile([NPC, D], F32)
        h0f = dr.tile([N, D], F32)
        for ci in range(NT):
            rows = min(P, NPC - ci * P)
            ga = wp.tile([P, D], F32)
            nc.gpsimd.indirect_dma_start(
                out=ga[:], out_offset=None, in_=tb[:],
                in_offset=bass.IndirectOffsetOnAxis(
                    ap=xpi[:, ci:ci + 1], axis=0))
            gb = wp.tile([P, D], F32)
            nc.gpsimd.indirect_dma_start(
                out=gb[:], out_offset=None, in_=tb[:],
                in_offset=bass.IndirectOffsetOnAxis(
                    ap=xpi[:, NT + ci:NT + ci + 1], axis=0))
            hs = wp.tile([P, D], F32)
            nc.vector.tensor_add(hs[:], ga[:], gb[:])
            nc.sync.dma_start(out=h0s[ci * P:ci * P + rows, :],
                              in_=hs[:rows, :])
        nc.gpsimd.collective_compute(
            "AllGather", mybir.AluOpType.bypass, replica_groups=RG,
            ins=[h0s.opt()], outs=[h0f.opt()])

        hf_prev = h0f
        for l in range(2):
            w1l = pf_sb[:, C_W1[l]:C_W1[l] + 2 * D]
            w2a = pf_sb[:, C_W2[2 * l]:C_W2[2 * l] + D]
            w2b = pf_sb[:, C_W2[2 * l + 1]:C_W2[2 * l + 1] + D]
            etab = pf_sb[:21, C_ET[l]:C_ET[l] + D]
            b1a = cp.tile([P, 1], F32, name=f"b1a{l}")
            nc.vector.tensor_copy(out=b1a[:],
                                  in_=pf_sb[:, C_B1 + 2 * l:C_B1 + 2 * l + 1])
            b1b = cp.tile([P, 1], F32, name=f"b1b{l}")
            nc.vector.tensor_copy(out=b1b[:],
                                  in_=pf_sb[:, C_B1 + 2 * l + 1:C_B1 + 2 * l + 2])

            s1a = cp.tile([1, D], F32, name=f"s1a{l}")
            s2a = cp.tile([1, D], F32, name=f"s2a{l}")
            nc.vector.memset(s1a[:], 0.0)
            nc.vector.memset(s2a[:], 0.0)
            for ti in range(NT):
                cols = min(P, NPC - ti * P)
                agg_ps = pa.tile([P, P], F32, space="PSUM")
                nc.tensor.matmul(
                    out=agg_ps[:, :cols], lhsT=etab,
                    rhs=cnf[:, ti * P:ti * P + cols],
                    start=True, stop=False, skip_group_check=True)
                for j in range(kt[ti]):
                    col = co[ti] + j
                    hg = wp.tile([P, D], F32)
                    nc.gpsimd.indirect_dma_start(
                        out=hg[:], out_offset=None, in_=hf_prev[:],
                        in_offset=bass.IndirectOffsetOnAxis(
                            ap=spi[:, col:col + 1], axis=0))
                    oh = wp.tile([P, P], F32)
                    nc.vector.tensor_tensor(
                        out=oh[:, :cols],
                        in0=dpf[:, col:col + 1].to_broadcast([P, cols]),
                        in1=iota[:, :cols], op=mybir.AluOpType.is_equal)
                    nc.tensor.matmul(
                        out=agg_ps[:, :cols], lhsT=hg[:], rhs=oh[:, :cols],
                        start=False, stop=(j == kt[ti] - 1),
                        skip_group_check=True)
                aggT = wp.tile([P, P], F32)
                nc.vector.tensor_copy(out=aggT[:, :cols], in_=agg_ps[:, :cols])

                # z = relu(W1^T @ agg + b1), feature-major halves in one tile
                z_ps = pb.tile([P, 2 * D], F32, space="PSUM")
                r = []
                for half, bsb in ((0, b1a[:]), (1, b1b[:])):
                    zs = z_ps[:, half * D:half * D + cols]
                    nc.tensor.matmul(
                        out=zs, lhsT=w1l[:, half * D:(half + 1) * D],
                        rhs=aggT[:, :cols], start=True, stop=True,
                        skip_group_check=True)
                    rh = wp.tile([P, P], F32)
                    nc.scalar.activation(
                        out=rh[:, :cols], in_=zs,
                        func=mybir.ActivationFunctionType.Relu, bias=bsb)
                    r.append(rh)

                # h2 rows: flip stationary/moving -> [cols, D] row-major
                h2_ps = pc.tile([P, P], F32, space="PSUM", name="h2p")
                nc.tensor.matmul(out=h2_ps[:cols, :], lhsT=r[0][:, :cols],
                                 rhs=w2a, start=True, stop=False,
                                 skip_group_check=True)
                nc.tensor.matmul(out=h2_ps[:cols, :], lhsT=r[1][:, :cols],
                                 rhs=w2b, start=False, stop=True,
                                 skip_group_check=True)
                hsl = h2R[l][:cols, ti * D:(ti + 1) * D]
                nc.vector.tensor_tensor(out=hsl, in0=h2_ps[:cols, :],
                                        in1=b2r[l][:cols, :],
                                        op=mybir.AluOpType.add)
                # BN stats: s1 += ones^T @ h2, s2 += ones^T @ h2^2
                sq = wp.tile([P, P], F32)
                nc.vector.tensor_mul(sq[:cols, :], hsl, hsl)
                s1t = pst.tile([1, D], F32, space="PSUM", name="s1t")
                s2t = pst.tile([1, D], F32, space="PSUM", name="s2t")
                nc.tensor.matmul(out=s1t[:], lhsT=ones[:cols, :], rhs=hsl,
                                 start=True, stop=True, skip_group_check=True)
                nc.tensor.matmul(out=s2t[:], lhsT=ones[:cols, :],
                                 rhs=sq[:cols, :],
                                 start=True, stop=True, skip_group_check=True)
                nc.vector.tensor_add(s1a[:], s1a[:], s1t[:])
                nc.vector.tensor_add(s2a[:], s2a[:], s2t[:])

            # stats all-reduce: [1, 256] row
            srow = cp.tile([1, 2 * D], F32, name=f"srow{l}")
            nc.vector.tensor_copy(out=srow[:, :D], in_=s1a[:])
            nc.vector.tensor_copy(out=srow[:, D:], in_=s2a[:])
            sb_d = dr.tile([1, 2 * D], F32)
            sr_d = dr.tile([1, 2 * D], F32)
            nc.gpsimd.dma_start(sb_d[:], srow[:])
            nc.gpsimd.collective_compute(
                "AllReduce", mybir.AluOpType.add, replica_groups=RG,
                ins=[sb_d.opt()], outs=[sr_d.opt()])
            sred = cp.tile([1, 2 * D], F32, name=f"sred{l}")
            nc.gpsimd.dma_start(sred[:], sr_d[:])

            # BN coeffs on [1, D] rows
            mu = cp.tile([1, D], F32, name=f"mu{l}")
            nc.vector.tensor_scalar_mul(mu[:], sred[:, :D], 1.0 / N)
            var = cp.tile([1, D], F32, name=f"var{l}")
            nc.vector.tensor_scalar_mul(var[:], sred[:, D:], 1.0 / N)
            mu2 = cp.tile([1, D], F32, name=f"mu2{l}")
            nc.vector.tensor_mul(mu2[:], mu[:], mu[:])
            nc.vector.tensor_tensor(out=var[:], in0=var[:], in1=mu2[:],
                                    op=mybir.AluOpType.subtract)
            nc.vector.tensor_scalar_add(var[:], var[:], BN_EPS)
            std = cp.tile([1, D], F32, name=f"std{l}")
            nc.scalar.activation(out=std[:], in_=var[:],
                                 func=mybir.ActivationFunctionType.Sqrt)
            rstd = cp.tile([1, D], F32, name=f"rstd{l}")
            nc.vector.reciprocal(out=rstd[:], in_=std[:])
            arow = cp.tile([1, D], F32, name=f"arow{l}")
            nc.vector.tensor_mul(arow[:], vr_sb[:, (2 + 2 * l) * D:(3 + 2 * l) * D],
                                 rstd[:])
            brow = cp.tile([1, D], F32, name=f"brow{l}")
            nc.vector.tensor_mul(brow[:], arow[:], mu[:])
            nc.vector.tensor_tensor(out=brow[:],
                                    in0=vr_sb[:, (3 + 2 * l) * D:(4 + 2 * l) * D],
                                    in1=brow[:], op=mybir.AluOpType.subtract)
            # replicate a/b rows to [128, D]
            reps = []
            for v in (arow, brow):
                rp = pc.tile([P, P], F32, space="PSUM", name="h2p")
                nc.tensor.matmul(out=rp[:], lhsT=onesr[:], rhs=v[:],
                                 start=True, stop=True, skip_group_check=True)
                rs = cp.tile([P, P], F32, name=f"rep{l}_{len(reps)}")
                nc.vector.tensor_copy(out=rs[:], in_=rp[:])
                reps.append(rs)
            ar, br = reps

            # BN apply on rows (+relu for layer 0), DMA rows out
            if l == 0:
                h1s = dr.tile([NPC, D], F32)
                h1f = dr.tile([N, D], F32)
                for ti in range(NT):
                    cols = min(P, NPC - ti * P)
                    y = wp.tile([P, D], F32)
                    nc.vector.tensor_mul(y[:cols, :],
                                         h2R[l][:cols, ti * D:(ti + 1) * D],
                                         ar[:cols, :])
                    nc.vector.tensor_tensor(out=y[:cols, :], in0=y[:cols, :],
                                            in1=br[:cols, :],
                                            op=mybir.AluOpType.add)
                    nc.vector.tensor_scalar_max(y[:cols, :], y[:cols, :], 0.0)
                    nc.sync.dma_start(out=h1s[ti * P:ti * P + cols, :],
                                      in_=y[:cols, :])
                nc.gpsimd.collective_compute(
                    "AllGather", mybir.AluOpType.bypass, replica_groups=RG,
                    ins=[h1s.opt()], outs=[h1f.opt()])
                hf_prev = h1f
            else:
                for ti in range(NT):
                    cols = min(P, NPC - ti * P)
                    y = wp.tile([P, D], F32)
                    nc.vector.tensor_mul(y[:cols, :],
                                         h2R[l][:cols, ti * D:(ti + 1) * D],
                                         ar[:cols, :])
                    nc.vector.tensor_tensor(out=y[:cols, :], in0=y[:cols, :],
                                            in1=br[:cols, :],
                                            op=mybir.AluOpType.add)
                    yb = wp.tile([P, D], BF16)
                    nc.vector.tensor_copy(out=yb[:cols, :], in_=y[:cols, :])
                    nc.sync.dma_start(out=outr[ti * P:ti * P + cols, :],
                                      in_=yb[:cols, :])
    nc.compile()
    return nc


LAUNCH_NS = []


def _run(nc, maps, cores):
    import time as _t
    t0 = _t.monotonic_ns()
    res = run_bass_kernel_spmd(nc, maps, cores)
    dt = _t.monotonic_ns() - t0
    LAUNCH_NS.append(res.exec_time_ns if res.exec_time_ns else dt)
    return res


def kernel(x, edge_index, edge_attr, batch, xemb1, xemb2, e1, e2,
           W1, b1, W2, b2, gamma, beta):
    LAUNCH_NS.clear()
    packed, kt, co, ct = _host_prep(x, edge_index, edge_attr)
    f32 = np.float32

    tb = np.concatenate([np.asarray(xemb1, f32)[:120],
                         np.asarray(xemb2, f32)], axis=0)
    pf = np.zeros((P, NF32), f32)
    pf[:, 0:128] = np.arange(P, dtype=f32)[None, :]
    vr = np.zeros((1, NVR), f32)
    for l in range(2):
        pf[:, C_W1[l]:C_W1[l] + 2 * D] = np.asarray(W1[l], f32)
        pf[:, C_W2[2 * l]:C_W2[2 * l] + D] = np.asarray(W2[l][:D], f32)
        pf[:, C_W2[2 * l + 1]:C_W2[2 * l + 1] + D] = np.asarray(W2[l][D:], f32)
        etab = (np.repeat(np.asarray(e1[l], f32), 3, axis=0)
                + np.tile(np.asarray(e2[l], f32), (7, 1)))
        pf[:21, C_ET[l]:C_ET[l] + D] = etab
        pf[:, C_B1 + 2 * l] = np.asarray(b1[l][:D], f32)
        pf[:, C_B1 + 2 * l + 1] = np.asarray(b1[l][D:], f32)
        vr[0, l * D:(l + 1) * D] = np.asarray(b2[l], f32)
        vr[0, (2 + 2 * l) * D:(3 + 2 * l) * D] = np.asarray(gamma[l], f32)
        vr[0, (3 + 2 * l) * D:(4 + 2 * l) * D] = np.asarray(beta[l], f32)

    nc = _build(kt, co, ct)
    maps = []
    for c in range(NCORES):
        m = {"tb": tb, "pf": pf, "vr": vr}
        m.update(packed[c])
        maps.append(m)
    res = _run(nc, maps, list(range(NCORES))).results
    return np.concatenate([r["o"].astype(np.float32) for r in res], axis=0)
